# revision 1
# baseline (speedup 1.0000x reference)
"""Trainium2 Bass kernel for nn_AlignCriterion (align loss).

Strategy: pure data-parallel over batch (8 batches per core, 8 cores).
The O(B*N^2*C) correlation/assignment einsums are algebraically collapsed:

  gclc_cor_loss = -0.15 * sum_b [ T1_b - T2_b ]
    T1_b = sum_{q,c} P[q,c] * R[q,c]
      P[q,c] = sum_n w~g[n,q] * gn[n,c]     (gn = normalized gc, w~ = masked softmax)
      R[q,c] = sum_m w~l[m,q] * ln[m,c]
    T2_b = sum_q (alpha'_q + (0.1 - g) * beta_q) * v_q
      alpha'_q = sum_n w~g[n,q] * r_n,  r_n = (gn[n,:] . s_lc)/784
      beta_q   = sum_n w~g[n,q],  v_q = sum_m w~l[m,q]
      g = sum_b (s_gc . s_lc) / (B*N*M)   (global scalar, combined on host)

  query CE loss computed per-batch from z = [q0n; q1n], sim = z z^T.

Per-core device output: [8, 128, 8] f32 partial sums; host combines in f64.
"""

import sys

import numpy as np

sys.path.insert(0, "/opt/trn_rl_repo")

import concourse.bass as bass  # noqa: E402,F401
import concourse.mybir as mybir  # noqa: E402
import concourse.tile as tile  # noqa: E402
from concourse import bacc  # noqa: E402
from concourse.bass_utils import run_bass_kernel_spmd  # noqa: E402
from concourse.masks import make_identity  # noqa: E402

F32 = mybir.dt.float32
BF16 = mybir.dt.bfloat16
I32 = mybir.dt.int32
AF = mybir.ActivationFunctionType
ALU = mybir.AluOpType
AX = mybir.AxisListType

B = 64
N = 784          # 28*28 spatial positions
C = 384
Q = 5
NCORES = 8
BL = B // NCORES  # batches per core = 8
NT = 7           # row tiles per batch: 6 full 128 + 1 tail of 16
TAIL = N - 6 * 128  # 16
NK = 3           # c chunks of 128

_CACHED = {}


def _build():
    nc = bacc.Bacc("TRN2", target_bir_lowering=False, debug=False,
                   num_devices=NCORES)

    gc_in = nc.dram_tensor("gc_in", [BL, N, C], F32, kind="ExternalInput").ap()
    lc_in = nc.dram_tensor("lc_in", [BL, N, C], F32, kind="ExternalInput").ap()
    q0_in = nc.dram_tensor("q0_in", [BL, Q, C], F32, kind="ExternalInput").ap()
    q1_in = nc.dram_tensor("q1_in", [BL, Q, C], F32, kind="ExternalInput").ap()
    att_in = nc.dram_tensor("att_in", [2 * BL, N], I32, kind="ExternalInput").ap()
    out = nc.dram_tensor("out", [BL, 128, 8], F32, kind="ExternalOutput").ap()

    with tile.TileContext(nc) as tc:
        _kernel(tc, out, gc_in, lc_in, q0_in, q1_in, att_in)

    # the installed walrus birverifier rejects EVENT_SEMAPHORE_RANGE_CLEAR
    # (opcode 176, emitted by the Tile kernel-tail sem cleanup). NRT re-inits
    # semaphores per execution, so drop the tail clear entirely.
    for fn in nc.m.functions:
        for blk in fn.blocks:
            il = blk.instructions
            for i in range(len(il) - 1, -1, -1):
                if isinstance(il[i], mybir.InstISA) and il[i].isa_opcode == 176:
                    del il[i]

    nc.compile()
    return nc


def _kernel(tc, out, gc_in, lc_in, q0_in, q1_in, att_in):
    from contextlib import ExitStack
    with ExitStack() as ctx:
        _kernel_inner(ctx, tc, out, gc_in, lc_in, q0_in, q1_in, att_in)


def _kernel_inner(ctx, tc, out, gc_in, lc_in, q0_in, q1_in, att_in):
    nc = tc.nc
    E = float(np.exp(1.0))

    consts = ctx.enter_context(tc.tile_pool(name="consts", bufs=1))
    sb = ctx.enter_context(tc.tile_pool(name="sb", bufs=2))
    sbs = ctx.enter_context(tc.tile_pool(name="sbs", bufs=3))
    ps_t = ctx.enter_context(tc.tile_pool(name="ps_t", bufs=2, space="PSUM"))
    ps_asg = ctx.enter_context(tc.tile_pool(name="ps_asg", bufs=2, space="PSUM"))
    ps_misc = ctx.enter_context(tc.tile_pool(name="ps_misc", bufs=2, space="PSUM"))
    ps_init = ctx.enter_context(tc.tile_pool(name="ps_init", bufs=1, space="PSUM"))

    # ---- constants ----
    ident_bf = consts.tile([128, 128], BF16, tag="ident_bf")
    make_identity(nc, ident_bf[:])
    ident_f = consts.tile([16, 16], F32, tag="ident_f")
    make_identity(nc, ident_f[:])
    # partner mask [10,10]: mask[r, (r+Q)%2Q] = 1
    pmask = consts.tile([10, 10], F32, tag="pmask")
    nc.gpsimd.memset(pmask[:], 0.0)
    nc.gpsimd.affine_select(out=pmask[:], in_=pmask[:], compare_op=ALU.not_equal,
                            fill=1.0, base=-Q, pattern=[[-1, 10]], channel_multiplier=1)
    nc.gpsimd.affine_select(out=pmask[:], in_=pmask[:], compare_op=ALU.not_equal,
                            fill=1.0, base=Q, pattern=[[-1, 10]], channel_multiplier=1)

    # ---- masks: att [16, 784] i32 -> U [128, 7, 16] f32 (col j = crop j) ----
    att_i = consts.tile([2 * BL, N], I32, tag="att_i")
    nc.sync.dma_start(att_i[:], att_in[:, :])
    att_f = consts.tile([2 * BL, N], F32, tag="att_f")
    nc.vector.tensor_copy(att_f[:], att_i[:])
    U = consts.tile([128, NT, 2 * BL], F32, tag="U")
    for t in range(NT):
        w = 128 if t < 6 else TAIL
        pt = ps_init.tile([128, 16], F32, tag="upsum")
        nc.tensor.transpose(pt[:w, :], att_f[:, t * 128:t * 128 + w], ident_f[:, :])
        nc.scalar.copy(U[:w, t, :], pt[:w, :])

    for b in range(BL):
        _batch(tc, b, out, gc_in, lc_in, q0_in, q1_in,
               sb, sbs, ps_t, ps_asg, ps_misc, ident_bf, pmask, U, E)


def _batch(tc, b, out, gc_in, lc_in, q0_in, q1_in,
           sb, sbs, ps_t, ps_asg, ps_misc, ident_bf, pmask, U, E):
    nc = tc.nc

    # ---------- load ----------
    gc_nat = sb.tile([128, NT, C], F32, tag="gc_nat")
    lc_nat = sb.tile([128, NT, C], F32, tag="lc_nat")
    for src, dst in ((gc_in, gc_nat), (lc_in, lc_nat)):
        nc.sync.dma_start(dst[:, 0:6, :],
                          src[b, 0:768, :].rearrange("(t p) c -> p t c", p=128))
        nc.sync.dma_start(dst[0:TAIL, 6, :], src[b, 768:N, :])

    q0 = sbs.tile([Q, C], F32, tag="q0")
    q1 = sbs.tile([Q, C], F32, tag="q1")
    nc.sync.dma_start(q0[:], q0_in[b])
    nc.sync.dma_start(q1[:], q1_in[b])

    # ---------- row stats: ss = sum_c x^2 ; inv = 1/max(sqrt(ss),1e-10) ----------
    ss = sbs.tile([128, 2 * NT + 2], F32, tag="ss")
    sq_scr = sbs.tile([128, C], BF16, tag="sq_scr")
    for t in range(NT):  # ACT Square with fused row-accumulate
        nc.scalar.activation(sq_scr[:], gc_nat[:, t, :], AF.Square,
                             accum_out=ss[:, t:t + 1])
    sq2 = sbs.tile([128, C], BF16, tag="sq2")
    for t in range(NT):
        nc.scalar.activation(sq2[:], lc_nat[:, t, :], AF.Square,
                             accum_out=ss[:, NT + t:NT + t + 1])
    qs_scr = sbs.tile([Q, C], BF16, tag="qs_scr")
    nc.scalar.activation(qs_scr[:], q0[:], AF.Square,
                         accum_out=ss[:Q, 2 * NT:2 * NT + 1])
    nc.scalar.activation(qs_scr[:], q1[:], AF.Square,
                         accum_out=ss[:Q, 2 * NT + 1:2 * NT + 2])

    inv = sbs.tile([128, 2 * NT + 2], F32, tag="inv")
    nc.scalar.sqrt(inv[:], ss[:])
    nc.vector.tensor_scalar_max(inv[:], inv[:], 1e-10)
    nc.vector.reciprocal(inv[:], inv[:])

    # ---------- normalized bf16 copies ----------
    gn = sb.tile([128, NT, C], BF16, tag="gn")
    ln = sb.tile([128, NT, C], BF16, tag="ln")
    for t in range(NT):
        nc.vector.tensor_scalar_mul(gn[:, t, :], gc_nat[:, t, :], inv[:, t:t + 1])
        nc.vector.tensor_scalar_mul(ln[:, t, :], lc_nat[:, t, :],
                                    inv[:, NT + t:NT + t + 1])
    q0n = sbs.tile([Q, C], BF16, tag="q0n")
    q1n = sbs.tile([Q, C], BF16, tag="q1n")
    nc.vector.tensor_scalar_mul(q0n[:], q0[:], inv[:Q, 2 * NT:2 * NT + 1])
    nc.vector.tensor_scalar_mul(q1n[:], q1[:], inv[:Q, 2 * NT + 1:2 * NT + 2])

    # ---------- zstag [128, 3, 16] bf16: [0:5]=q0nT, [5]=slcN, [6:11]=q0nT, [11:16]=q1nT ----------
    zstag = sbs.tile([128, NK, 16], BF16, tag="zstag")
    for k in range(NK):
        pt = ps_t.tile([128, 4, 128], BF16, tag="tpsum")
        nc.tensor.transpose(pt[:, 0, 0:Q], q0n[:, k * 128:(k + 1) * 128],
                            ident_bf[:Q, :Q])
        nc.tensor.transpose(pt[:, 1, 0:Q], q1n[:, k * 128:(k + 1) * 128],
                            ident_bf[:Q, :Q])
        nc.scalar.copy(zstag[:, k, 0:Q], pt[:, 0, 0:Q])
        nc.scalar.copy(zstag[:, k, 6:6 + Q], pt[:, 0, 0:Q])
        nc.scalar.copy(zstag[:, k, 11:16], pt[:, 1, 0:Q])

    # misc psum: pr [128,2,3,8] at cols 0:48, abv [16,4] at 48:52, sim [10,10] at 52:62
    misc = ps_misc.tile([128, 64], F32, tag="misc")
    pr = misc[:, 0:48].rearrange("p (i k e) -> p i k e", i=2, k=NK)
    abv = misc[:16, 48:52]
    sim = misc[:10, 52:62]
    rn_ones = sbs.tile([128, NT, 2], BF16, tag="rn_ones")
    nc.gpsimd.memset(rn_ones[:], 1.0)

    # ---------- per-tensor phase: lc first (produces slcN), then gc ----------
    for side in ("lc", "gc"):
        xn = ln if side == "lc" else gn
        crop = (BL + b) if side == "lc" else b
        pri = 0 if side == "gc" else 1

        # -- transpose xn -> staging [128c, 3k, 784n] bf16 --
        stag = sb.tile([128, NK, N], BF16, tag=f"stag_{side}")
        for k in range(NK):
            for half, (t0, nth) in enumerate(((0, 4), (4, 3))):
                pt = ps_t.tile([128, 4, 128], BF16, tag="tpsum")
                for j in range(nth):
                    t = t0 + j
                    w = 128 if t < 6 else TAIL
                    nc.tensor.transpose(pt[:, j, 0:w],
                                        xn[0:w, t, k * 128:(k + 1) * 128],
                                        ident_bf[0:w, 0:w])
                eng = nc.vector if (k + half) % 2 == 0 else nc.scalar
                copy = (eng.tensor_copy if eng is nc.vector else eng.copy)
                if t0 + nth <= 6:
                    copy(stag[:, k, t0 * 128:(t0 + nth) * 128].rearrange(
                        "p (j w) -> p j w", j=nth), pt[:, 0:nth, :])
                else:
                    copy(stag[:, k, t0 * 128:(t0 + nth - 1) * 128].rearrange(
                        "p (j w) -> p j w", j=nth - 1), pt[:, 0:nth - 1, :])
                    copy(stag[:, k, 6 * 128:6 * 128 + TAIL],
                         pt[:, nth - 1, 0:TAIL])

        # -- assignment logits: asg [128, 7, 8] f32; gc also gets r col 5 --
        ncol = 6 if side == "gc" else Q
        rcols = slice(0, 6) if side == "gc" else slice(11, 16)
        asg = ps_asg.tile([128, NT, 8], F32, tag="asg")
        for t in range(NT):
            w = 128 if t < 6 else TAIL
            for k in range(NK):
                nc.tensor.matmul(asg[0:w, t, 0:ncol],
                                 stag[:, k, t * 128:t * 128 + w],
                                 zstag[:, k, rcols],
                                 start=(k == 0), stop=(k == NK - 1))

        # -- softmax over Q with relu; w~ = e * (u/sumexp) --
        e_t = sbs.tile([128, NT, Q], F32, tag=f"e_{side}")
        nc.vector.tensor_scalar_max(e_t[:], asg[:, :, 0:Q], 0.0)
        nc.scalar.activation(e_t[:], e_t[:], AF.Exp)
        sume = sbs.tile([128, NT], F32, tag=f"sume_{side}")
        nc.vector.tensor_reduce(sume[:], e_t[:], axis=AX.X, op=ALU.add)
        nc.vector.reciprocal(sume[:], sume[:])
        stil = sbs.tile([128, NT], F32, tag=f"stil_{side}")
        nc.vector.tensor_tensor(out=stil[:], in0=sume[:], in1=U[:, :, crop],
                                op=ALU.mult)
        wt = sbs.tile([128, NT, 6], BF16, tag=f"wt_{side}")
        nc.gpsimd.memset(wt[:, :, Q:6], 1.0 if side == "gc" else 1.0 / N)
        for t in range(NT):
            nc.vector.tensor_scalar_mul(wt[:, t, 0:Q], e_t[:, t, :],
                                        stil[:, t:t + 1])

        # -- P/R (+ s col): accumulate over tiles --
        for k in range(NK):
            for t in range(NT):
                w = 128 if t < 6 else TAIL
                nc.tensor.matmul(pr[:, pri, k, 0:6],
                                 xn[0:w, t, k * 128:(k + 1) * 128],
                                 wt[0:w, t, :],
                                 start=(t == 0), stop=(t == NT - 1))

        if side == "lc":
            for k in range(NK):  # slcN -> zstag col 5 (bf16)
                nc.scalar.copy(zstag[:, k, Q:Q + 1], pr[:, 1, k, 5:6])
            for t in range(NT):  # vq
                w = 128 if t < 6 else TAIL
                nc.tensor.matmul(abv[:Q, 2:3], wt[0:w, t, 0:Q],
                                 rn_ones[0:w, t, 0:1],
                                 start=(t == 0), stop=(t == NT - 1))
        else:
            nc.vector.tensor_copy(rn_ones[:, :, 0], asg[:, :, 5])
            for t in range(NT):  # alpha', beta
                w = 128 if t < 6 else TAIL
                nc.tensor.matmul(abv[:Q, 0:2], wt[0:w, t, 0:Q],
                                 rn_ones[0:w, t, :],
                                 start=(t == 0), stop=(t == NT - 1))

    # ---------- query CE ----------
    for k in range(NK):
        nc.tensor.matmul(sim[:, :], zstag[:, k, 6:16], zstag[:, k, 6:16],
                         start=(k == 0), stop=(k == NK - 1))
    esum = sbs.tile([10, 4], F32, tag="esum")
    esim = sbs.tile([10, 10], F32, tag="esim")
    nc.scalar.activation(esim[:], sim[:, :], AF.Exp, accum_out=esum[:, 0:1])
    nc.vector.tensor_scalar_add(esum[:, 1:2], esum[:, 0:1], -E)
    nc.scalar.activation(esum[:, 2:3], esum[:, 1:2], AF.Ln)
    pos_scr = sbs.tile([10, 10], F32, tag="pos_scr")
    nc.vector.tensor_tensor(out=pos_scr[:], in0=sim[:, :], in1=pmask[:],
                            op=ALU.mult)
    nc.vector.tensor_reduce(esum[:, 3:4], pos_scr[:], axis=AX.X, op=ALU.add)
    ce = sbs.tile([10, 1], F32, tag="ce")
    nc.vector.tensor_tensor(out=ce[:], in0=esum[:, 2:3], in1=esum[:, 3:4],
                            op=ALU.subtract)

    # ---------- batch partials -> out[b] ----------
    ot = sbs.tile([128, 8], F32, tag="ot")
    nc.gpsimd.memset(ot[:], 0.0)
    # TensorTensor may read only one input from PSUM: drain P (gc side) to SBUF
    psb = sbs.tile([128, NK, 6], F32, tag="psb")
    nc.scalar.copy(psb[:], pr[:, 0, :, 0:6])
    t1_scr = sbs.tile([128, NK, Q], F32, tag="t1_scr")
    nc.vector.tensor_tensor(out=t1_scr[:], in0=psb[:, :, 0:Q],
                            in1=pr[:, 1, :, 0:Q], op=ALU.mult)
    nc.vector.tensor_reduce(ot[:, 0:1], t1_scr[:], axis=AX.XY, op=ALU.add)
    g_scr = sbs.tile([128, NK], F32, tag="g_scr")
    nc.vector.tensor_tensor(out=g_scr[:], in0=psb[:, :, 5],
                            in1=pr[:, 1, :, 5], op=ALU.mult)
    nc.vector.tensor_reduce(ot[:, 1:2], g_scr[:], axis=AX.X, op=ALU.add)
    nc.scalar.copy(ot[:10, 2:3], ce[:])
    nc.scalar.copy(ot[:Q, 3:6], abv[:Q, 0:3])
    nc.sync.dma_start(out[b], ot[:])


def _combine(results):
    T1 = 0.0
    G = 0.0
    ce_sum = 0.0
    abv = []
    for r in results:
        o = np.asarray(r["out"], dtype=np.float64)  # [BL, 128, 8]
        T1 += o[:, :, 0].sum()
        G += o[:, :, 1].sum()
        ce_sum += o[:, :10, 2].sum()
        abv.append(o[:, :Q, 3:6])
    abv = np.concatenate(abv, 0)  # [B, Q, 3] : alpha', beta, vq
    g = G / (B * N)
    T2 = ((abv[:, :, 0] + (0.1 - g) * abv[:, :, 1]) * abv[:, :, 2]).sum()
    loss1 = -0.15 * (T1 - T2)
    loss2 = ce_sum / (B * 2 * Q)
    return np.float32(loss1 + loss2)


def kernel(all_queries_0, all_queries_1, gc_output, lc_output, attn_hard,
           gc_spatial_res=None, lc_spatial_res=None):
    if "nc" not in _CACHED:
        _CACHED["nc"] = _build()
    nc = _CACHED["nc"]

    gc = np.ascontiguousarray(np.asarray(gc_output, dtype=np.float32))
    lc = np.ascontiguousarray(np.asarray(lc_output, dtype=np.float32)[:, 0])
    q0 = np.ascontiguousarray(np.asarray(all_queries_0, dtype=np.float32))
    q1 = np.ascontiguousarray(np.asarray(all_queries_1, dtype=np.float32))
    att = np.asarray(attn_hard, dtype=np.int32).reshape(2 * B, N)

    in_maps = []
    for i in range(NCORES):
        s = slice(i * BL, (i + 1) * BL)
        in_maps.append({
            "gc_in": gc[s],
            "lc_in": lc[s],
            "q0_in": q0[s],
            "q1_in": q1[s],
            "att_in": np.ascontiguousarray(
                np.concatenate([att[s], att[B + i * BL:B + (i + 1) * BL]], 0)),
        })
    res = run_bass_kernel_spmd(nc, in_maps, core_ids=list(range(NCORES)))
    return _combine(res.results)

